# revision 25
# baseline (speedup 1.0000x reference)
"""CapsNet dynamic-routing kernel for 8 Trainium2 NeuronCores.

Strategy: shard n_routes (4096) across 8 cores (512 routes/core).
All bulk data is bf16 (tolerance 2e-2; measured end-to-end ~3e-3):
  - u_hat produced once per core via PE matmuls (block-diagonal x as
    stationary, host-pre-transposed W[(rb,i),(o,c)] as moving) and kept
    resident in SBUF as [p=(rb,b), (g,o,c)] bf16 (64 KB/partition).
    c innermost (stride 1) keeps every big DVE tensor_tensor in 2x_1p.
  - s0 (uniform-softmax route sum) via an interleaved DVE pair-tree
    over groups + one delta matmul (no per-group PE work).
  - routing iterations: DVE mults (u*v_rep, u*e) in bf16; PE does the
    o-reduction + batch-mean in one PSUM-accumulated pass (a-phase)
    and the delta route-sum (s-phase); per-region b+=/exp so the
    s-phase overlaps the a-phase PE tail.
  - the per-iteration AllReduce payload is laid out [128,96] f32
    (s re-partitioned over (b, o-pair), softmax denominator replicated
    8x) so the squash runs on all 128 partitions (~8x fewer cycles),
    with direct PSUM->DRAM DMAs on the send side.
  - squash computed as v = s*|s| / (dn^2 + s^2).
"""

import numpy as np

B, R, I, C, O = 16, 4096, 16, 32, 16
NCORES = 8
RL = R // NCORES      # 512 routes per core
G = RL // 8           # 64 groups of 8 routes
CO = C * O            # 512
GPB = 8               # groups per DMA block
NBLK = G // GPB       # 8 blocks
SEG = CO + 128        # per-group stage: wt(512) + xb(128)
CHG = 16              # groups per DVE chunk
NCH = G // CHG        # 8 chunks
GPR = 16              # groups per a-phase PSUM region

_cache = {}


def _patch_tile_drain():
    import concourse.tile as tile_mod
    from concourse.vector_clock import ScopedClock, VectorClock

    if getattr(tile_mod.TileContext, "_drain_patched", False):
        return

    def _split_drain_and_barrier(self, tick_clock, wait_clock):
        ticks = list(tick_clock.global_clock)
        for i in [j for j, t in enumerate(ticks) if t > 0]:
            vec = [ticks[j] if j == i else 0 for j in range(len(ticks))]
            d = self.nc.sync.drain()
            wait_clock.add_sem_waits(d.ins, ScopedClock({None: VectorClock(vec)}))
        self.nc.all_engine_barrier()
        popped = self.nc._tile_sem_poison_stack.pop()
        assert popped is self._sem_poison
        self.nc.clear_and_free_semaphores(list(self.sems.allocated().values()))
        self.nc.all_engine_barrier()

    tile_mod.TileContext._drain_and_barrier = _split_drain_and_barrier
    tile_mod.TileContext._drain_patched = True


def _split_waits(nc, limit=1):
    """This container's walrus rejects >1 sync-wait per instruction; move
    excess waits onto same-engine NoOps inserted just before the owner."""
    import concourse.mybir as mybir

    blocks = nc.main_func.blocks
    for bb in blocks:
        insts = bb.instructions  # live list view
        k = 0
        while k < len(insts):
            inst = insts[k]
            si = inst.sync_info
            if si is not None and si.on_wait and len(si.on_wait) > limit:
                w = list(si.on_wait)
                si.on_wait = w[:limit]
                excess = w[limit:]
                insert_at = k
                for cs in range(0, len(excess), limit):
                    chunk = excess[cs:cs + limit]
                    nop = nc.engines[inst.engine].nop()
                    ni = nop.ins
                    for bb2 in blocks:
                        l2 = bb2.instructions
                        hit = next(
                            (i for i in range(len(l2) - 1, -1, -1)
                             if l2[i].name == ni.name), None)
                        if hit is not None:
                            l2.pop(hit)
                            break
                    ni.sync_info = mybir.SyncInfo(on_wait=chunk, on_update=[])
                    insts.insert(insert_at, ni)
                    insert_at += 1
                    k += 1
            k += 1


def _build_nc():
    import concourse.bass as bass
    import concourse.mybir as mybir
    from concourse.tile import TileContext

    _patch_tile_drain()
    F32 = mybir.dt.float32
    BF16 = mybir.dt.bfloat16
    AF = mybir.ActivationFunctionType
    ALU = mybir.AluOpType
    CORES = list(range(NCORES))

    nc = bass.Bass(target_bir_lowering=False)
    wx_d = nc.declare_dram_parameter("wx", [NBLK, 128, GPB * SEG], BF16,
                                     isOutput=False)
    db_d = nc.declare_dram_parameter("delta_b", [128, 16], BF16, isOutput=False)
    dbs_d = nc.declare_dram_parameter("delta_bs", [128, 16], BF16,
                                      isOutput=False)
    ob_d = nc.declare_dram_parameter("ones_bd", [128, 128], BF16, isOutput=False)
    o16_d = nc.declare_dram_parameter("ones_16", [128, 16], F32, isOutput=False)
    dr_d = nc.declare_dram_parameter("delta_rep", [16, 128], BF16, isOutput=False)
    out_d = nc.declare_dram_parameter("out", [16, CO], F32, isOutput=True)
    cc_in = [nc.dram_tensor(f"cc_in{k}", [128, 96], F32) for k in range(3)]
    cc_out = [
        nc.dram_tensor(f"cc_out{k}", [128, 96], F32, addr_space="Shared")
        for k in range(3)
    ]
    vd = nc.dram_tensor("vd", [16, CO], BF16)
    ccd_in = nc.dram_tensor("ccd_in", [128, 96], F32)
    ccd_out = nc.dram_tensor("ccd_out", [128, 96], F32, addr_space="Shared")


    with TileContext(nc) as tc:
        with (
            tc.tile_pool(name="big", bufs=1) as big,
            tc.tile_pool(name="stage", bufs=3) as stage,
            tc.tile_pool(name="small", bufs=1) as small,
            tc.tile_pool(name="psA", bufs=4, space="PSUM") as psA,
            tc.tile_pool(name="psS", bufs=1, space="PSUM") as psS,
            tc.tile_pool(name="psD", bufs=1, space="PSUM") as psD,
            tc.tile_pool(name="psAB", bufs=2, space="PSUM") as psAB,
        ):
            # constants
            db = small.tile([128, 16], BF16, tag="db")
            dbs = small.tile([128, 16], BF16, tag="dbs")
            ob = small.tile([128, 128], BF16, tag="ob")
            o16 = small.tile([128, 16], F32, tag="o16")
            dr = small.tile([16, 128], BF16, tag="dr")
            nc.sync.dma_start(out=db[:, :], in_=db_d[:, :])
            nc.sync.dma_start(out=dbs[:, :], in_=dbs_d[:, :])
            nc.sync.dma_start(out=ob[:, :], in_=ob_d[:, :])
            nc.sync.dma_start(out=o16[:, :], in_=o16_d[:, :])
            nc.sync.dma_start(out=dr[:, :], in_=dr_d[:, :])

            u_sb = big.tile([128, G, O, C], BF16, tag="u")
            prod = big.tile([128, G, O, C], BF16, tag="prod")
            b_f = small.tile([128, G, C], F32, tag="b_f")
            e16 = small.tile([128, G, C], BF16, tag="e16")
            v_rep = small.tile([128, O, C], BF16, tag="v_rep")
            dnscr = small.tile([128, 32, C], F32, tag="dnscr")
            dn8 = small.tile([16, 8, C], F32, tag="dn8")
            # dummy collective: the CC channel init is async from exec
            # start (~78us); a first tiny CC absorbs the expensive
            # first-collective path under the production phase
            nc.gpsimd.collective_compute(
                "AllReduce", ALU.add, replica_groups=[CORES],
                ins=[ccd_in[:, :]], outs=[ccd_out[:, :]],
            )
            nc.vector.memset(b_f[:, :, :], 0.0)

            # ---- production; s0 via interleaved DVE pair-tree ----
            DMAQ = [nc.sync, nc.gpsimd, nc.scalar]
            for blk in range(NBLK):
                stg = stage.tile([128, GPB * SEG], BF16, tag="stg")
                DMAQ[blk % 3].dma_start(out=stg[:, :], in_=wx_d[blk, :, :])
                for j in range(GPB):
                    g = blk * GPB + j
                    base = j * SEG
                    wt = stg[:, base:base + CO]
                    xb = stg[:, base + CO:base + SEG]
                    pu = psA.tile([128, CO], F32, tag="pu")
                    nc.tensor.matmul(pu[:, :], xb, wt, start=True, stop=True)
                    puv = pu[:, :].rearrange("p (o c) -> p o c", o=O)
                    if g % 2 == 0:
                        nc.vector.tensor_copy(u_sb[:, g, :, :], puv)
                    else:
                        nc.scalar.copy(u_sb[:, g, :, :], puv)
                # block pair-tree: prod[8k:8k+4] = u[8k:8k+4] + u[8k+4:8k+8]
                k8 = blk * GPB
                nc.vector.tensor_tensor(
                    prod[:, k8:k8 + 4], u_sb[:, k8:k8 + 4],
                    u_sb[:, k8 + 4:k8 + 8], ALU.add,
                )
                if blk % 2 == 1:
                    p0 = (blk - 1) * GPB
                    nc.vector.tensor_tensor(
                        prod[:, p0:p0 + 4], prod[:, p0:p0 + 4],
                        prod[:, p0 + 8:p0 + 12], ALU.add,
                    )
            for p0, p1 in ((0, 16), (32, 48), (0, 32)):
                nc.vector.tensor_tensor(
                    prod[:, p0:p0 + 4], prod[:, p0:p0 + 4],
                    prod[:, p1:p1 + 4], ALU.add,
                )
            nc.vector.tensor_tensor(
                prod[:, 0:2], prod[:, 0:2], prod[:, 2:4], ALU.add)
            nc.vector.tensor_tensor(
                prod[:, 0:1], prod[:, 0:1], prod[:, 1:2], ALU.add)
            ps_s = psS.tile([16, CO], F32, tag="ps_s")
            nc.tensor.matmul(ps_s[:, :], dbs[:, :], prod[:, 0, :, :],
                             start=True, stop=True)

            st_s = small.tile([16, CO], F32, tag="st_s")

            def start_cc(it, with_dn):
                # s part: [16,(o,c)] -> DRAM [(b,oh), (ol,c)]
                nc.scalar.copy(st_s[:, :], ps_s[:, :])
                nc.sync.dma_start(
                    out=cc_in[it][:, :].rearrange(
                        "(b oh) f -> b oh f", b=16)[:, :, 0:64],
                    in_=st_s[:, :].rearrange("p (oh f) -> p oh f", oh=8),
                )
                if with_dn:
                    nc.scalar.dma_start(
                        out=cc_in[it][:, :].rearrange(
                            "(b oh) f -> b oh f", b=16)[:, :, 64:96],
                        in_=dn8[:, :, :],
                    )
                nc.gpsimd.collective_compute(
                    "AllReduce", ALU.add, replica_groups=[CORES],
                    ins=[cc_in[it][:, :]], outs=[cc_out[it][:, :]],
                )
                nc.sync.dma_start(out=st2[:, :], in_=cc_out[it][:, :])

            st2 = small.tile([128, 96], F32, tag="st2")
            start_cc(0, with_dn=False)

            sq = small.tile([128, 64], F32, tag="sq")
            den = small.tile([128, 64], F32, tag="den")
            m1 = small.tile([128, 64], F32, tag="m1")
            v128 = small.tile([128, 64], BF16, tag="v128")
            v128f = small.tile([128, 64], F32, tag="v128f")
            dn2 = small.tile([128, C], F32, tag="dn2")
            ab = small.tile([128, 64], F32, tag="ab")
            v16b = small.tile([16, CO], BF16, tag="v16b")

            def squash(k):
                # v = squash(s/dn) = s*|s| / (dn^2 + s^2);  dn==1 for k==0
                s = st2[:, 0:64]
                nc.scalar.activation(ab[:, :], s, AF.Abs)
                nc.vector.tensor_tensor(m1[:, :], s, ab[:, :], ALU.mult)
                nc.vector.tensor_tensor(sq[:, :], s, s, ALU.mult)
                if k == 0:
                    nc.vector.tensor_scalar_add(den[:, :], sq[:, :], 1.0)
                else:
                    d = st2[:, 64:96]
                    nc.vector.tensor_tensor(dn2[:, :], d, d, ALU.mult)
                    nc.vector.tensor_tensor(
                        den[:, :].rearrange("p (ol c) -> p ol c", ol=2),
                        sq[:, :].rearrange("p (ol c) -> p ol c", ol=2),
                        dn2[:, :].unsqueeze(1).broadcast_to([128, 2, C]),
                        ALU.add,
                    )
                nc.vector.reciprocal(den[:, :], den[:, :])
                out = v128f if k == 2 else v128
                nc.vector.tensor_tensor(out[:, :], m1[:, :], den[:, :], ALU.mult)

            for it in (1, 2):
                squash(it - 1)
                # repartition v to [16, (o,c)] and replicate over rb
                nc.scalar.dma_start(out=v16b[:, :], in_=v128[:, :])
                ps_vr = psA.tile([128, CO], F32, tag="pu")
                nc.tensor.matmul(ps_vr[:, :], dr[:, :], v16b[:, :],
                                 start=True, stop=True)
                nc.scalar.copy(
                    v_rep[:, :, :],
                    ps_vr[:, :].rearrange("p (o c) -> p o c", o=O),
                )

                # ---- a-phase: prod = u*v_rep (DVE); PE o-sum + batch-mean
                #      fused via ob; per-chunk b+= and exp so the s-phase
                #      can start early
                for ch in range(NCH):
                    gs = ch * CHG
                    nc.vector.tensor_tensor(
                        prod[:, gs:gs + CHG],
                        u_sb[:, gs:gs + CHG],
                        v_rep[:, :, :].unsqueeze(1).broadcast_to(
                            [128, CHG, O, C]),
                        ALU.mult,
                    )
                    ps_ab = psAB.tile([128, CHG, C], F32, tag="ps_ab")
                    for o in range(O):
                        nc.tensor.matmul(
                            ps_ab[:, :, :],
                            ob[:, :],
                            prod[:, gs:gs + CHG, o, :],
                            start=(o == 0), stop=(o == O - 1),
                        )
                    nc.vector.tensor_tensor(
                        b_f[:, gs:gs + CHG, :], b_f[:, gs:gs + CHG, :],
                        ps_ab[:, :, :], ALU.add,
                    )
                    nc.scalar.activation(
                        e16[:, gs:gs + CHG, :], b_f[:, gs:gs + CHG, :],
                        AF.Exp,
                    )

                # ---- s-phase: prod = u*e (DVE); PE delta-sums routes.
                #      tapered chunks: short final chunks shrink the PE tail
                #      before the collective
                g = 0
                for ci, w in enumerate((16, 16, 16, 8, 8)):
                    gs = g
                    nc.vector.tensor_tensor(
                        prod[:, gs:gs + w],
                        u_sb[:, gs:gs + w],
                        e16[:, gs:gs + w, :].unsqueeze(2).broadcast_to(
                            [128, w, O, C]),
                        ALU.mult,
                    )
                    for j in range(w):
                        nc.tensor.matmul(
                            ps_s[:, :], db[:, :], prod[:, gs + j, :, :],
                            start=(gs + j == 0), stop=(gs + j == G - 1),
                        )
                    g += w
                    if ci == 2:
                        # softmax denominator: f32 pair-tree over groups
                        # (DVE is ~8us ahead of the PE backlog here)
                        nc.vector.tensor_tensor(
                            dnscr[:, :, :], e16[:, 0:32, :], e16[:, 32:64, :],
                            ALU.add)
                        for w2 in (16, 8, 4, 2, 1):
                            nc.vector.tensor_tensor(
                                dnscr[:, 0:w2], dnscr[:, 0:w2],
                                dnscr[:, w2:2 * w2], ALU.add)
                        ps_dn = psD.tile([16, C], F32, tag="ps_dn")
                        nc.tensor.matmul(ps_dn[:, :], o16[:, :],
                                         dnscr[:, 0, :],
                                         start=True, stop=True)
                        nc.scalar.copy(
                            dn8[:, :, :],
                            ps_dn[:, :].unsqueeze(1).broadcast_to([16, 8, C]),
                        )
                start_cc(it, with_dn=True)

            squash(2)
            nc.sync.dma_start(out=out_d[:, :], in_=v128f[:, :])

    _split_waits(nc)
    return nc


def _prep_inputs(x, W):
    import ml_dtypes

    BF = ml_dtypes.bfloat16
    x = np.ascontiguousarray(x, np.float32)
    W = np.ascontiguousarray(W, np.float32)
    # wt[core, g, (rb,i), (o,c)] = W[r=(core,g,rb), c, o, i]
    Wv = W.reshape(NCORES, G, 8, C, O, I)
    wt = np.ascontiguousarray(
        Wv.transpose(0, 1, 2, 5, 4, 3).reshape(NCORES, G, 128, CO)
    ).astype(BF)
    # xv[core, g, rb, i, b] = x[b, r, i]
    xv = np.ascontiguousarray(x.transpose(1, 2, 0)).reshape(NCORES, G, 8, I, B)
    xb = np.zeros((NCORES, G, 128, 128), np.float32)
    for rb in range(8):
        xb[:, :, rb * 16:(rb + 1) * 16, rb * 16:(rb + 1) * 16] = xv[:, :, rb]
    xb = xb.astype(BF)
    wx = np.zeros((NCORES, NBLK, 128, GPB * SEG), BF)
    for j in range(GPB):
        base = j * SEG
        wx[:, :, :, base:base + CO] = wt.reshape(NCORES, NBLK, GPB, 128, CO)[:, :, j]
        wx[:, :, :, base + CO:base + SEG] = \
            xb.reshape(NCORES, NBLK, GPB, 128, 128)[:, :, j]
    db = np.tile(np.eye(16, dtype=np.float32), (8, 1))               # [128,16]
    dbs = (db / np.float32(R)).astype(BF)
    db = db.astype(BF)
    ob = np.kron(np.eye(8, dtype=np.float32),
                 np.full((16, 16), 1.0 / B, np.float32)).astype(BF)  # [128,128]
    o16 = np.full((128, 16), 1.0 / 16.0, np.float32)
    dr = np.tile(np.eye(16, dtype=np.float32), (1, 8)).astype(BF)    # [16,128]
    in_maps = []
    for c in range(NCORES):
        in_maps.append({
            "wx": wx[c],
            "delta_b": db, "delta_bs": dbs, "ones_bd": ob,
            "ones_16": o16, "delta_rep": dr,
        })
    return in_maps


def kernel(x, W):
    from concourse.bass_utils import run_bass_kernel_spmd

    if "nc" not in _cache:
        _cache["nc"] = _build_nc()
    in_maps = _prep_inputs(x, W)
    res = run_bass_kernel_spmd(_cache["nc"], in_maps, list(range(NCORES)))
    # out is [b, (o,c)] -> reference layout [b, c, o, 1]
    v = res.results[0]["out"].reshape(B, O, C).transpose(0, 2, 1)[..., None]
    return np.ascontiguousarray(v, np.float32)
